# revision 44
# baseline (speedup 1.0000x reference)
"""Multi-head attention (B=2, S=2048, D=1024, H=16 heads, causal) on 8 TRN2
NeuronCores, head-parallel: each core computes 2 heads' Q/K/V projections,
attention, and a partial output projection (its 128-row slice of Wo); the
host sums the 8 partial outputs (bf16 partials, fp32 sum).

Per-core layout (matmul operands bf16, fp32 PSUM accumulation):
  - xt [128,8,8,512]     x^T pre-tiled on host as [partition, strip, k, col]
  - wq/wk/wv [128,8,128] per-core column slice of Wq/Wk/Wv, pre-tiled
  - wo [128, 1024]       per-core row slice of Wo
  - msk [128, 2, 128]    upper-triangular keep mask, duplicated for 2 heads
  - idn [128, 128]       identity for PE transposes of the V projection
  QT/KT are produced transposed [128 = 2 heads x 64 head dims, 4096 tokens];
  V is stored per (b, h, kv-tile) as [128 kv, 64] plus an appended ones
  column so the attention matmul also accumulates softmax denominators
  (row 64 of the [65, 512] PSUM output).

Attention is emitted at kv-tile granularity: for each 128-kv tile the two
heads' score matmuls (K=64) are adjacent so they run concurrently in the
upper/lower halves of the PE array (row tiling); one 1024-wide exp covers
both heads. Projection/output-projection units are interleaved between
tile units so the PE never idles long enough to re-throttle.
"""

import numpy as np
import ml_dtypes
from contextlib import ExitStack

import concourse.bass as bass
import concourse.bacc as bacc
import concourse.tile as tile
import concourse.mybir as mybir
from concourse.bass_utils import run_bass_kernel_spmd

BF16 = mybir.dt.bfloat16
F32 = mybir.dt.float32
FP8 = mybir.dt.float8e4
NPBF16 = ml_dtypes.bfloat16
NPFP8 = ml_dtypes.float8_e4m3fn
FP8_QK = False  # fp8 DoubleRow Q/K projections: fast but 3.2% err (gate is 2%)
WSCALE = 32.0 if FP8_QK else 1.0  # fp8 Wq/Wk pre-scale; folded into exp scale

D = 1024          # model dim
B = 2
S = 2048
NT = B * S        # 4096 flattened tokens
HD = 64           # head dim
H = 16            # total heads
NCORES = 8
HLOC = H // NCORES  # 2 heads per core
CW = HLOC * HD      # 128 local columns
QSTRIP = 512
NSTRIP = NT // QSTRIP  # 8 strips
KT_TILES = S // 128    # 16 kv tiles per batch


def _interleave(main, fill):
    """Emit main units with fill units spread proportionally between them."""
    n, m = len(main), len(fill)
    if n == 0:
        for u in fill:
            u()
        return
    fi = 0
    for i, u in enumerate(main):
        u()
        tgt = ((i + 1) * m) // n
        while fi < tgt:
            fill[fi]()
            fi += 1
    while fi < m:
        fill[fi]()
        fi += 1


def _build_kernel(ctx: ExitStack, tc: tile.TileContext):
    nc = tc.nc
    # pre-arranged on host: xt[p, strip, k, col], w*[p, k, col]
    xt = nc.dram_tensor("xt", [128, NSTRIP, 8, QSTRIP], BF16,
                        kind="ExternalInput").ap()
    if FP8_QK:
        xq = nc.dram_tensor("xq", [128, NSTRIP, 4, 2, QSTRIP], FP8,
                            kind="ExternalInput").ap()
        wq = nc.dram_tensor("wq", [128, 4, 2, CW], FP8,
                            kind="ExternalInput").ap()
        wk = nc.dram_tensor("wk", [128, 4, 2, CW], FP8,
                            kind="ExternalInput").ap()
    else:
        wq = nc.dram_tensor("wq", [128, 8, CW], BF16,
                            kind="ExternalInput").ap()
        wk = nc.dram_tensor("wk", [128, 8, CW], BF16,
                            kind="ExternalInput").ap()
    wv = nc.dram_tensor("wv", [128, 8, CW], BF16, kind="ExternalInput").ap()
    wo = nc.dram_tensor("wo", [CW, D], BF16, kind="ExternalInput").ap()
    msk = nc.dram_tensor("msk", [128, 2, 128], BF16, kind="ExternalInput").ap()
    idn = nc.dram_tensor("idn", [128, 128], BF16, kind="ExternalInput").ap()
    out = nc.dram_tensor("out", [NT, D], BF16, kind="ExternalOutput").ap()

    singles = ctx.enter_context(tc.tile_pool(name="singles", bufs=1))
    sbp = ctx.enter_context(tc.tile_pool(name="sbp", bufs=3))
    expp = ctx.enter_context(tc.tile_pool(name="expp", bufs=6))
    outp = ctx.enter_context(tc.tile_pool(name="outp", bufs=3))
    rbp = ctx.enter_context(tc.tile_pool(name="rbp", bufs=2))
    psM = ctx.enter_context(tc.tile_pool(name="psM", bufs=2, space="PSUM"))
    psS = ctx.enter_context(tc.tile_pool(name="psS", bufs=2, space="PSUM"))
    psV = ctx.enter_context(tc.tile_pool(name="psV", bufs=2, space="PSUM"))

    # --- staging: small weights first, then xT strip-major so strip 0's
    # projection can start early while later strips stream in behind it.
    # fp8 copies of x and Wq/Wk (scaled x32) feed the Q/K DoubleRow matmuls;
    # bf16 x feeds the V projection.
    w_sb = {}
    xt_sb = singles.tile([128, 8, NT], BF16)
    ones_sb = singles.tile([HD + 1, HD], BF16)
    nc.vector.memset(ones_sb, 1.0)
    # warm-up matmuls on local constants: keeps the PE busy during the
    # initial DMA wait so HAM unthrottles before the real stream begins
    wrm = singles.tile([HD, QSTRIP], BF16, tag="wrm", name="wrm")
    nc.vector.memset(wrm, 0.01)
    wps = psM.tile([HD, QSTRIP], F32, tag="mm", name="warm_ps")
    for i in range(14):
        nc.tensor.matmul(wps, lhsT=ones_sb[0:HD, :], rhs=wrm,
                         start=(i == 0), stop=(i == 13))
    if FP8_QK:
        xq_sb = singles.tile([128, 4, 2, NT], FP8)

    def load_xt(g):
        gs = g * QSTRIP
        eng = nc.sync if g % 2 == 0 else nc.gpsimd
        eng.dma_start(out=xt_sb[:, :, gs:gs + QSTRIP], in_=xt[:, g, :, :])

    def load_xq(g):
        if not FP8_QK:
            return
        gs = g * QSTRIP
        nc.gpsimd.dma_start(out=xq_sb[:, :, :, gs:gs + QSTRIP],
                            in_=xq[:, g, :, :, :])

    if FP8_QK:
        w_sb["q"] = singles.tile([128, 4, 2, CW], FP8, tag="wq8", name="w_q")
        nc.gpsimd.dma_start(out=w_sb["q"], in_=wq)
        load_xq(0)
        w_sb["k"] = singles.tile([128, 4, 2, CW], FP8, tag="wk8", name="w_k")
        nc.gpsimd.dma_start(out=w_sb["k"], in_=wk)
        load_xt(0)
    else:
        # strip 0 in k-pair chunks spread over queues (including the
        # otherwise-idle vector/scalar queues) so the first projection
        # matmul can start as soon as wq + chunk 0 land
        w_sb["q"] = singles.tile([128, 8, CW], BF16, tag="wq", name="w_q")
        nc.scalar.dma_start(out=w_sb["q"], in_=wq)
        nc.sync.dma_start(out=xt_sb[:, 0:2, 0:QSTRIP], in_=xt[:, 0, 0:2, :])
        w_sb["k"] = singles.tile([128, 8, CW], BF16, tag="wk", name="w_k")
        nc.gpsimd.dma_start(out=w_sb["k"], in_=wk)
        nc.sync.dma_start(out=xt_sb[:, 2:4, 0:QSTRIP], in_=xt[:, 0, 2:4, :])
        nc.gpsimd.dma_start(out=xt_sb[:, 4:6, 0:QSTRIP], in_=xt[:, 0, 4:6, :])
        nc.sync.dma_start(out=xt_sb[:, 6:8, 0:QSTRIP], in_=xt[:, 0, 6:8, :])
    load_xt(1)
    w_sb["v"] = singles.tile([128, 8, CW], BF16, tag="wv", name="w_v")
    nc.gpsimd.dma_start(out=w_sb["v"], in_=wv)
    msk_sb = singles.tile([128, 2, 128], BF16)
    nc.gpsimd.dma_start(out=msk_sb, in_=msk)
    idn_sb = singles.tile([128, 128], BF16)
    nc.gpsimd.dma_start(out=idn_sb, in_=idn)
    for g in range(1, NSTRIP):
        load_xq(g)
    wo_sb = singles.tile([128, D], BF16)
    nc.gpsimd.dma_start(out=wo_sb, in_=wo)
    for g in range(2, NSTRIP):
        load_xt(g)

    qt_sb = singles.tile([128, NT], BF16)
    kt_sb = singles.tile([128, NT], BF16)
    v_sb = singles.tile([128, B * HLOC * KT_TILES, HD + 1], BF16)
    nc.vector.memset(v_sb[:, :, HD:HD + 1], 1.0)

    # dedicated (non-pooled) avT tensors per strip: no WAR hazards
    avf = {g: singles.tile([128, QSTRIP], BF16, tag=f"avf{g}", name=f"avf{g}")
           for g in range(NSTRIP)}

    def proj_units(g):
        gs = g * QSTRIP
        st = {}

        def qk_mm(name, lo, hi, first, last, dst):
            def u():
                if first:
                    st[name] = psM.tile([128, QSTRIP], F32, tag="mm", name=f"ps_{name}")
                ps = st[name]
                if FP8_QK:
                    for kg in range(lo // 2, hi // 2):
                        nc.tensor.matmul(
                            ps, lhsT=w_sb[name][:, kg, :, :],
                            rhs=xq_sb[:, kg, :, gs:gs + QSTRIP],
                            start=(kg == 0), stop=(kg == 3),
                            perf_mode=mybir.MatmulPerfMode.DoubleRow)
                else:
                    for k in range(lo, hi):
                        nc.tensor.matmul(
                            ps, lhsT=w_sb[name][:, k, :],
                            rhs=xt_sb[:, k, gs:gs + QSTRIP],
                            start=(k == 0), stop=(k == 7))
                if last:
                    nc.any.tensor_copy(dst[:, gs:gs + QSTRIP], ps)
            return u

        groups = {"q": [qk_mm("q", 0, 4, True, False, qt_sb),
                        qk_mm("q", 4, 8, False, True, qt_sb)],
                  "k": [qk_mm("k", 0, 4, True, False, kt_sb),
                        qk_mm("k", 4, 8, False, True, kt_sb)]}

        b, j = divmod(g, 4)

        def v_mm(tlo, thi, first, last):
            def u():
                if first:
                    st["v"] = psM.tile([128, 4, 128], F32, tag="mm",
                                       name="ps_v")
                ps = st["v"]
                for tt in range(tlo, thi):
                    for k in range(8):
                        nc.tensor.matmul(
                            ps[:, tt, :],
                            lhsT=xt_sb[:, k,
                                       gs + tt * 128: gs + (tt + 1) * 128],
                            rhs=w_sb["v"][:, k, :],
                            start=(k == 0), stop=(k == 7))
                if last:
                    v4 = v_sb.rearrange("p (b h t) c -> p b h t c",
                                        b=B, h=HLOC)
                    nc.any.tensor_copy(
                        v4[:, b, :, 4 * j: 4 * j + 4, 0:HD],
                        ps.rearrange("p t (h d) -> p h t d", h=HLOC))
            return u

        groups["v"] = [v_mm(0, 2, True, False), v_mm(2, 4, False, True)]
        return groups

    def attn_units(g, fine_tail=False):
        """Per-kv-tile units: scores for both heads adjacent (row-tiled
        concurrency), one wide exp, triangular mask on diagonal tiles,
        then both heads' AV matmuls. Scores run 2 tiles ahead."""
        b, j = divmod(g, 4)
        T = 4 * (j + 1)
        st = {}

        def q0_of(t):
            return max(0, 128 * (t - 4 * j))

        def mk_sc(t):
            def u():
                sc = psS.tile([128, 2, QSTRIP], F32, tag="sc", name="sc_ps")
                st[t] = sc
                q0 = q0_of(t)
                for h in range(HLOC):
                    hp = h * HD
                    nc.tensor.matmul(
                        sc[:, h, q0:],
                        lhsT=kt_sb[hp:hp + HD,
                                   b * S + t * 128: b * S + (t + 1) * 128],
                        rhs=qt_sb[hp:hp + HD,
                                  b * S + j * QSTRIP + q0:
                                  b * S + (j + 1) * QSTRIP],
                        start=True, stop=True)
            return u

        def pexp_sl(pexp, t, h, qlo, qhi):
            return pexp[:, h, qlo:qhi]

        def mk_exp(t):
            sc = st.pop(t)
            q0 = q0_of(t)
            pexp = expp.tile([128, 2, QSTRIP], BF16, tag="pexp",
                             name="pexp")
            nc.scalar.activation(
                pexp[:, :, q0:], sc[:, :, q0:],
                mybir.ActivationFunctionType.Exp,
                scale=0.125 / (WSCALE * WSCALE))
            if t >= 4 * j:  # diagonal block: triangular mask at q0
                nc.vector.tensor_mul(
                    pexp[:, :, q0:q0 + 128], pexp[:, :, q0:q0 + 128],
                    msk_sb)
            return pexp

        def mk_ea(t):
            def u():
                q0 = q0_of(t)
                if t == 0:
                    st["av0"] = psV.tile([HD + 1, QSTRIP], F32, tag="av",
                                         name="av0_ps")
                    st["av1"] = psV.tile([HD + 1, QSTRIP], F32, tag="av",
                                         name="av1_ps")
                pexp = mk_exp(t)
                for h in range(HLOC):
                    idx = (b * HLOC + h) * KT_TILES + t
                    nc.tensor.matmul(
                        st[f"av{h}"][:, q0:], lhsT=v_sb[:, idx, :],
                        rhs=pexp_sl(pexp, t, h, q0, QSTRIP),
                        start=(t == 0), stop=(t == T - 1))
            return u

        def mk_norm():
            def u():
                # copy AV out of PSUM fast (releases the banks); the
                # denominator row is broadcast across partitions by a tiny
                # K=1 PE matmul against a ones column, then inverted on DVE
                av_sb = sbp.tile([HD + 1, 2, QSTRIP], BF16, tag="avsb",
                                 name="av_sb")
                db = [None, None]
                for h in range(HLOC):
                    if h == 0:
                        nc.vector.tensor_copy(av_sb[:, h, :], st[f"av{h}"])
                    else:
                        nc.scalar.copy(av_sb[:, h, :], st[f"av{h}"])
                    db[h] = psM.tile([HD, QSTRIP], F32, tag="mm",
                                     name=f"db{h}")
                    nc.tensor.matmul(db[h],
                                     lhsT=ones_sb[HD:HD + 1, :],
                                     rhs=av_sb[HD:HD + 1, h, :],
                                     start=True, stop=True)
                rb = rbp.tile([HD, 2, QSTRIP], F32, tag="rb")
                for h in range(HLOC):
                    nc.vector.reciprocal_approx_fast(rb[:, h, :], db[h])
                nc.vector.tensor_mul(avf[g][0:HD, :], av_sb[0:HD, 0, :],
                                     rb[:, 0, :])
                avh = sbp.tile([HD, QSTRIP], BF16, tag="avh")
                nc.vector.tensor_mul(avh, av_sb[0:HD, 1, :], rb[:, 1, :])
                nc.gpsimd.dma_start(out=avf[g][HD:2 * HD, :], in_=avh)
            return u

        def mk_exp_fine(t):
            # last strip: just exp+mask; AV deferred to per-q-block units
            def u():
                if t == 0:
                    for h in range(HLOC):
                        st[f"avq{h}"] = psV.tile([HD + 1, 4, 128], F32,
                                                 tag="av", name=f"avq{h}")
                st[f"pexp{t}"] = mk_exp(t)
            return u

        def mk_av_fine(qb):
            # one contiguous accumulation group per q-block (interleaved
            # open groups in one PSUM bank corrupt results)
            def u():
                for h in range(HLOC):
                    for t in range(qb + 1):
                        idx = (b * HLOC + h) * KT_TILES + t
                        nc.tensor.matmul(
                            st[f"avq{h}"][:, qb, :], lhsT=v_sb[:, idx, :],
                            rhs=pexp_sl(st[f"pexp{t}"], t, h, qb * 128,
                                        (qb + 1) * 128),
                            start=(t == 0), stop=(t == qb))
            return u

        def mk_norm_fine(qb):
            def u():
                av_sbq = sbp.tile([HD + 1, 2, 128], BF16, tag="avsbq",
                                  name="av_sbq")
                for h in range(HLOC):
                    nc.vector.tensor_copy(av_sbq[:, h, :],
                                          st[f"avq{h}"][:, qb, :])
                db = psM.tile([HD, 2, 128], F32, tag="mm", name="dbq")
                for h in range(HLOC):
                    nc.tensor.matmul(db[:, h, :],
                                     lhsT=ones_sb[HD:HD + 1, :],
                                     rhs=av_sbq[HD:HD + 1, h, :],
                                     start=True, stop=True)
                rbq = rbp.tile([HD, 2, 128], F32, tag="rbq")
                nc.vector.reciprocal_approx_fast(rbq, db)
                qs = qb * 128
                nc.vector.tensor_mul(avf[g][0:HD, qs:qs + 128],
                                     av_sbq[0:HD, 0, :], rbq[:, 0, :])
                avhq = sbp.tile([HD, 128], BF16, tag="avhq")
                nc.vector.tensor_mul(avhq, av_sbq[0:HD, 1, :], rbq[:, 1, :])
                nc.gpsimd.dma_start(out=avf[g][HD:2 * HD, qs:qs + 128],
                                    in_=avhq)
            return u

        if fine_tail:
            assert T == 4
            outu = out_units(g)
            units = [mk_sc(0), mk_sc(1)]
            for t in range(T):
                units.append(mk_exp_fine(t))
                if t + 2 < T:
                    units.append(mk_sc(t + 2))
                units.append(mk_av_fine(t))
                units.append(mk_norm_fine(t))
                units.append(outu[t])
            return units

        units = [mk_sc(0), mk_sc(1)]
        for t in range(T):
            units.append(mk_ea(t))
            if t + 2 < T:
                units.append(mk_sc(t + 2))
        units.append(mk_norm())
        return units

    def out_units(g):
        gs = g * QSTRIP
        units = []
        def mk(tt):
            def u():
                ob = outp.tile([128, D], BF16, tag="ob")
                for n in range(2):
                    op_ps = psM.tile([128, 512], F32, tag="mm", name="op_ps")
                    nc.tensor.matmul(
                        op_ps, lhsT=avf[g][:, tt * 128:(tt + 1) * 128],
                        rhs=wo_sb[:, n * 512:(n + 1) * 512],
                        start=True, stop=True)
                    # gap-filler copy: Tile routes to whichever of ACT/DVE
                    # is idle when it becomes ready
                    nc.any.tensor_copy(ob[:, n * 512:(n + 1) * 512], op_ps)
                nc.sync.dma_start(
                    out=out[gs + tt * 128: gs + (tt + 1) * 128, :], in_=ob)
            return u
        for tt in range(4):
            units.append(mk(tt))
        return units

    # strip order: b0 ascending then b1 descending (short strip last).
    order = [0, 1, 2, 3, 7, 6, 5, 4]
    pu = {g: proj_units(g) for g in range(NSTRIP)}

    def pf(g, keys):
        return [u for k in keys for u in pu[g][k]]

    for u in pf(0, "qkv"):
        u()

    fill_sched = {
        0: lambda: pf(1, "qkv"),
        1: lambda: pf(2, "qkv") + out_units(0),
        2: lambda: pf(3, "qkv") + pf(4, "kv") + out_units(1),
        3: lambda: pf(7, "qkv") + pf(6, "kv") + pf(5, "kv")
                   + out_units(2),
        7: lambda: pf(6, "q"),
        6: lambda: out_units(3) + pf(5, "q"),
        5: lambda: out_units(7) + pf(4, "q"),
        4: None,
    }
    for g in order[:-1]:
        _interleave(attn_units(g), fill_sched[g]())
    ou5 = out_units(5)
    _interleave(attn_units(4), out_units(6) + ou5[0:1])
    # dummy warm-fill matmuls: execute during the final norm-chain stalls so
    # the HAM clock stays at full rate for the drain's real matmuls
    wfill = psV.tile([HD, QSTRIP], F32, tag="av", name="wfill")
    ou4 = out_units(4)
    tail = [ou5[1], ou5[2], ou5[3]] + ou4
    for i, u in enumerate(tail):
        for r in range(2):
            nc.tensor.matmul(wfill, lhsT=ones_sb[0:HD, :], rhs=wrm,
                             start=(i == 0 and r == 0),
                             stop=(i == len(tail) - 1 and r == 1))
        u()


_CACHED_NC = None


def build_module():
    global _CACHED_NC
    if _CACHED_NC is None:
        nc = bacc.Bacc("TRN2", debug=False)
        with tile.TileContext(nc) as tc:
            with ExitStack() as ctx:
                _build_kernel(ctx, tc)
        nc.compile()
        _CACHED_NC = nc
    return _CACHED_NC


def make_in_maps(x, Wq, Wk, Wv, Wo):
    x = np.asarray(x, np.float32)
    xTf = x.reshape(NT, D).T                        # [D, NT] fp32
    # device layout [p, strip, k, col]: row d = k*128 + p
    xT4 = xTf.reshape(8, 128, NSTRIP, QSTRIP).transpose(1, 2, 0, 3)
    xT = np.ascontiguousarray(xT4.astype(NPBF16))
    if FP8_QK:
        # fp8 copy for Q/K: [p, strip, kg, ko, col] with k = 2*kg + ko
        xq = np.ascontiguousarray(
            xT4.reshape(128, NSTRIP, 4, 2, QSTRIP).astype(NPFP8))
    # triangular keep mask for the diagonal 128-block, duplicated per head
    i = np.arange(128)[:, None]
    c = np.arange(128)[None, :]
    tri = (c >= i).astype(NPBF16)
    msk = np.ascontiguousarray(
        np.stack([tri, tri], axis=1))               # [128, 2, 128]
    in_maps = []
    for core in range(NCORES):
        cs = slice(core * CW, (core + 1) * CW)
        def warr8(W):  # [D, CW] -> [p, kg, ko, col], scaled x32, fp8
            a = np.asarray(W, np.float32)[:, cs] * WSCALE
            return np.ascontiguousarray(
                a.reshape(4, 2, 128, CW).transpose(2, 0, 1, 3).astype(NPFP8))
        def warr(W):  # [D, CW] -> [p, k, col] with d = k*128 + p
            a = np.asarray(W, np.float32)[:, cs].astype(NPBF16)
            return np.ascontiguousarray(
                a.reshape(8, 128, CW).transpose(1, 0, 2))
        m = {
            "xt": xT,
            "wq": warr8(Wq) if FP8_QK else warr(Wq),
            "wk": warr8(Wk) if FP8_QK else warr(Wk),
            "wv": warr(Wv),
            "wo": np.ascontiguousarray(np.asarray(Wo, np.float32)[cs, :]).astype(NPBF16),
            "msk": msk,
            "idn": np.eye(128, dtype=NPBF16),
        }
        if FP8_QK:
            m["xq"] = xq
        in_maps.append(m)
    return in_maps


def kernel(x, Wq, bq, Wk, bk, Wv, bv, Wo, bo):
    for b_ in (bq, bk, bv, bo):
        assert np.count_nonzero(np.asarray(b_)) == 0, "nonzero biases unsupported"
    nc = build_module()
    in_maps = make_in_maps(x, Wq, Wk, Wv, Wo)
    res = run_bass_kernel_spmd(nc, in_maps, core_ids=list(range(NCORES)))
    partials = [res.results[c]["out"] for c in range(NCORES)]
    total = np.sum(np.stack(partials, 0).astype(np.float32), axis=0)
    return total.reshape(B, S, D)


# revision 45
# speedup vs baseline: 1.1631x; 1.1631x over previous
"""Multi-head attention (B=2, S=2048, D=1024, H=16 heads, causal) on 8 TRN2
NeuronCores, head-parallel: each core computes 2 heads' Q/K/V projections,
attention, and a partial output projection (its 128-row slice of Wo); the
host sums the 8 partial outputs (bf16 partials, fp32 sum).

Per-core layout (matmul operands bf16, fp32 PSUM accumulation):
  - xt [128,8,8,512]     x^T pre-tiled on host as [partition, strip, k, col]
  - wq/wk/wv [128,8,128] per-core column slice of Wq/Wk/Wv, pre-tiled
  - wo [128, 1024]       per-core row slice of Wo
  - msk [128, 2, 128]    upper-triangular keep mask, duplicated for 2 heads
  - idn [128, 128]       identity for PE transposes of the V projection
  QT/KT are produced transposed [128 = 2 heads x 64 head dims, 4096 tokens];
  V is stored per (b, h, kv-tile) as [128 kv, 64] plus an appended ones
  column so the attention matmul also accumulates softmax denominators
  (row 64 of the [65, 512] PSUM output).

Attention is emitted at kv-tile granularity: for each 128-kv tile the two
heads' score matmuls (K=64) are adjacent so they run concurrently in the
upper/lower halves of the PE array (row tiling); one 1024-wide exp covers
both heads. Projection/output-projection units are interleaved between
tile units so the PE never idles long enough to re-throttle.
"""

import numpy as np
import ml_dtypes
from contextlib import ExitStack

import concourse.bass as bass
import concourse.bacc as bacc
import concourse.tile as tile
import concourse.mybir as mybir
from concourse.bass_utils import run_bass_kernel_spmd

BF16 = mybir.dt.bfloat16
F32 = mybir.dt.float32
FP8 = mybir.dt.float8e4
NPBF16 = ml_dtypes.bfloat16
NPFP8 = ml_dtypes.float8_e4m3fn
FP8_QK = False  # fp8 DoubleRow Q/K projections: fast but 3.2% err (gate is 2%)
WSCALE = 32.0 if FP8_QK else 1.0  # fp8 Wq/Wk pre-scale; folded into exp scale

D = 1024          # model dim
B = 2
S = 2048
NT = B * S        # 4096 flattened tokens
HD = 64           # head dim
H = 16            # total heads
NCORES = 8
HLOC = H // NCORES  # 2 heads per core
CW = HLOC * HD      # 128 local columns
QSTRIP = 512
NSTRIP = NT // QSTRIP  # 8 strips
KT_TILES = S // 128    # 16 kv tiles per batch


def _interleave(main, fill):
    """Emit main units with fill units spread proportionally between them."""
    n, m = len(main), len(fill)
    if n == 0:
        for u in fill:
            u()
        return
    fi = 0
    for i, u in enumerate(main):
        u()
        tgt = ((i + 1) * m) // n
        while fi < tgt:
            fill[fi]()
            fi += 1
    while fi < m:
        fill[fi]()
        fi += 1


def _build_kernel(ctx: ExitStack, tc: tile.TileContext):
    nc = tc.nc
    # pre-arranged on host: xt[p, strip, k, col], w*[p, k, col]
    xt = nc.dram_tensor("xt", [128, NSTRIP, 8, QSTRIP], BF16,
                        kind="ExternalInput").ap()
    if FP8_QK:
        xq = nc.dram_tensor("xq", [128, NSTRIP, 4, 2, QSTRIP], FP8,
                            kind="ExternalInput").ap()
        wq = nc.dram_tensor("wq", [128, 4, 2, CW], FP8,
                            kind="ExternalInput").ap()
        wk = nc.dram_tensor("wk", [128, 4, 2, CW], FP8,
                            kind="ExternalInput").ap()
    else:
        wq = nc.dram_tensor("wq", [128, 8, CW], BF16,
                            kind="ExternalInput").ap()
        wk = nc.dram_tensor("wk", [128, 8, CW], BF16,
                            kind="ExternalInput").ap()
    wv = nc.dram_tensor("wv", [128, 8, CW], BF16, kind="ExternalInput").ap()
    wo = nc.dram_tensor("wo", [CW, D], BF16, kind="ExternalInput").ap()
    msk = nc.dram_tensor("msk", [128, 2, 128], BF16, kind="ExternalInput").ap()
    idn = nc.dram_tensor("idn", [128, 128], BF16, kind="ExternalInput").ap()
    out = nc.dram_tensor("out", [NT, D], BF16, kind="ExternalOutput").ap()

    singles = ctx.enter_context(tc.tile_pool(name="singles", bufs=1))
    sbp = ctx.enter_context(tc.tile_pool(name="sbp", bufs=3))
    expp = ctx.enter_context(tc.tile_pool(name="expp", bufs=6))
    outp = ctx.enter_context(tc.tile_pool(name="outp", bufs=3))
    rbp = ctx.enter_context(tc.tile_pool(name="rbp", bufs=2))
    psM = ctx.enter_context(tc.tile_pool(name="psM", bufs=2, space="PSUM"))
    psS = ctx.enter_context(tc.tile_pool(name="psS", bufs=2, space="PSUM"))
    psV = ctx.enter_context(tc.tile_pool(name="psV", bufs=2, space="PSUM"))

    # --- staging: small weights first, then xT strip-major so strip 0's
    # projection can start early while later strips stream in behind it.
    # fp8 copies of x and Wq/Wk (scaled x32) feed the Q/K DoubleRow matmuls;
    # bf16 x feeds the V projection.
    w_sb = {}
    xt_sb = singles.tile([128, 8, NT], BF16)
    ones_sb = singles.tile([HD + 1, HD], BF16)
    nc.vector.memset(ones_sb, 1.0)
    # warm-up matmuls on local constants: keeps the PE busy during the
    # initial DMA wait so HAM unthrottles before the real stream begins
    wrm = singles.tile([HD, QSTRIP], BF16, tag="wrm", name="wrm")
    nc.vector.memset(wrm, 0.01)
    wps = psM.tile([HD, QSTRIP], F32, tag="mm", name="warm_ps")
    for i in range(14):
        nc.tensor.matmul(wps, lhsT=ones_sb[0:HD, :], rhs=wrm,
                         start=(i == 0), stop=(i == 13))
    if FP8_QK:
        xq_sb = singles.tile([128, 4, 2, NT], FP8)

    def load_xt(g):
        gs = g * QSTRIP
        eng = nc.sync if g % 2 == 0 else nc.gpsimd
        eng.dma_start(out=xt_sb[:, :, gs:gs + QSTRIP], in_=xt[:, g, :, :])

    def load_xq(g):
        if not FP8_QK:
            return
        gs = g * QSTRIP
        nc.gpsimd.dma_start(out=xq_sb[:, :, :, gs:gs + QSTRIP],
                            in_=xq[:, g, :, :, :])

    if FP8_QK:
        w_sb["q"] = singles.tile([128, 4, 2, CW], FP8, tag="wq8", name="w_q")
        nc.gpsimd.dma_start(out=w_sb["q"], in_=wq)
        load_xq(0)
        w_sb["k"] = singles.tile([128, 4, 2, CW], FP8, tag="wk8", name="w_k")
        nc.gpsimd.dma_start(out=w_sb["k"], in_=wk)
        load_xt(0)
    else:
        # strip 0 in k-pair chunks spread over queues (including the
        # otherwise-idle vector/scalar queues) so the first projection
        # matmul can start as soon as wq + chunk 0 land
        w_sb["q"] = singles.tile([128, 8, CW], BF16, tag="wq", name="w_q")
        nc.scalar.dma_start(out=w_sb["q"], in_=wq)
        nc.sync.dma_start(out=xt_sb[:, 0:2, 0:QSTRIP], in_=xt[:, 0, 0:2, :])
        w_sb["k"] = singles.tile([128, 8, CW], BF16, tag="wk", name="w_k")
        nc.gpsimd.dma_start(out=w_sb["k"], in_=wk)
        nc.sync.dma_start(out=xt_sb[:, 2:4, 0:QSTRIP], in_=xt[:, 0, 2:4, :])
        nc.gpsimd.dma_start(out=xt_sb[:, 4:6, 0:QSTRIP], in_=xt[:, 0, 4:6, :])
        nc.sync.dma_start(out=xt_sb[:, 6:8, 0:QSTRIP], in_=xt[:, 0, 6:8, :])
    load_xt(1)
    w_sb["v"] = singles.tile([128, 8, CW], BF16, tag="wv", name="w_v")
    nc.gpsimd.dma_start(out=w_sb["v"], in_=wv)
    msk_sb = singles.tile([128, 2, 128], BF16)
    nc.gpsimd.dma_start(out=msk_sb, in_=msk)
    idn_sb = singles.tile([128, 128], BF16)
    nc.gpsimd.dma_start(out=idn_sb, in_=idn)
    for g in range(1, NSTRIP):
        load_xq(g)
    wo_sb = singles.tile([128, D], BF16)
    nc.gpsimd.dma_start(out=wo_sb, in_=wo)
    for g in range(2, NSTRIP):
        load_xt(g)

    qt_sb = singles.tile([128, NT], BF16)
    kt_sb = singles.tile([128, NT], BF16)
    v_sb = singles.tile([128, B * HLOC * KT_TILES, HD + 1], BF16)
    nc.vector.memset(v_sb[:, :, HD:HD + 1], 1.0)

    # dedicated (non-pooled) avT tensors per strip: no WAR hazards
    avf = {g: singles.tile([128, QSTRIP], BF16, tag=f"avf{g}", name=f"avf{g}")
           for g in range(NSTRIP)}

    def proj_units(g):
        gs = g * QSTRIP
        st = {}

        def qk_mm(name, lo, hi, first, last, dst):
            def u():
                if first:
                    st[name] = psM.tile([128, QSTRIP], F32, tag="mm", name=f"ps_{name}")
                ps = st[name]
                if FP8_QK:
                    for kg in range(lo // 2, hi // 2):
                        nc.tensor.matmul(
                            ps, lhsT=w_sb[name][:, kg, :, :],
                            rhs=xq_sb[:, kg, :, gs:gs + QSTRIP],
                            start=(kg == 0), stop=(kg == 3),
                            perf_mode=mybir.MatmulPerfMode.DoubleRow)
                else:
                    for k in range(lo, hi):
                        nc.tensor.matmul(
                            ps, lhsT=w_sb[name][:, k, :],
                            rhs=xt_sb[:, k, gs:gs + QSTRIP],
                            start=(k == 0), stop=(k == 7))
                if last:
                    nc.vector.tensor_copy(dst[:, gs:gs + QSTRIP], ps)
            return u

        groups = {"q": [qk_mm("q", 0, 4, True, False, qt_sb),
                        qk_mm("q", 4, 8, False, True, qt_sb)],
                  "k": [qk_mm("k", 0, 4, True, False, kt_sb),
                        qk_mm("k", 4, 8, False, True, kt_sb)]}

        b, j = divmod(g, 4)

        def v_mm(tlo, thi, first, last):
            def u():
                if first:
                    st["v"] = psM.tile([128, 4, 128], F32, tag="mm",
                                       name="ps_v")
                ps = st["v"]
                for tt in range(tlo, thi):
                    for k in range(8):
                        nc.tensor.matmul(
                            ps[:, tt, :],
                            lhsT=xt_sb[:, k,
                                       gs + tt * 128: gs + (tt + 1) * 128],
                            rhs=w_sb["v"][:, k, :],
                            start=(k == 0), stop=(k == 7))
                if last:
                    v4 = v_sb.rearrange("p (b h t) c -> p b h t c",
                                        b=B, h=HLOC)
                    nc.vector.tensor_copy(
                        v4[:, b, :, 4 * j: 4 * j + 4, 0:HD],
                        ps.rearrange("p t (h d) -> p h t d", h=HLOC))
            return u

        groups["v"] = [v_mm(0, 2, True, False), v_mm(2, 4, False, True)]
        return groups

    def attn_units(g, fine_tail=False):
        """Per-kv-tile units: scores for both heads adjacent (row-tiled
        concurrency), one wide exp, triangular mask on diagonal tiles,
        then both heads' AV matmuls. Scores run 2 tiles ahead."""
        b, j = divmod(g, 4)
        T = 4 * (j + 1)
        st = {}

        def q0_of(t):
            return max(0, 128 * (t - 4 * j))

        def mk_sc(t):
            def u():
                sc = psS.tile([128, 2, QSTRIP], F32, tag="sc", name="sc_ps")
                st[t] = sc
                q0 = q0_of(t)
                for h in range(HLOC):
                    hp = h * HD
                    nc.tensor.matmul(
                        sc[:, h, q0:],
                        lhsT=kt_sb[hp:hp + HD,
                                   b * S + t * 128: b * S + (t + 1) * 128],
                        rhs=qt_sb[hp:hp + HD,
                                  b * S + j * QSTRIP + q0:
                                  b * S + (j + 1) * QSTRIP],
                        start=True, stop=True)
            return u

        def pexp_sl(pexp, t, h, qlo, qhi):
            return pexp[:, h, qlo:qhi]

        def mk_exp(t):
            sc = st.pop(t)
            q0 = q0_of(t)
            pexp = expp.tile([128, 2, QSTRIP], BF16, tag="pexp",
                             name="pexp")
            nc.scalar.activation(
                pexp[:, :, q0:], sc[:, :, q0:],
                mybir.ActivationFunctionType.Exp,
                scale=0.125 / (WSCALE * WSCALE))
            if t >= 4 * j:  # diagonal block: triangular mask at q0
                nc.vector.tensor_mul(
                    pexp[:, :, q0:q0 + 128], pexp[:, :, q0:q0 + 128],
                    msk_sb)
            return pexp

        def mk_ea(t):
            def u():
                q0 = q0_of(t)
                if t == 0:
                    st["av0"] = psV.tile([HD + 1, QSTRIP], F32, tag="av",
                                         name="av0_ps")
                    st["av1"] = psV.tile([HD + 1, QSTRIP], F32, tag="av",
                                         name="av1_ps")
                pexp = mk_exp(t)
                for h in range(HLOC):
                    idx = (b * HLOC + h) * KT_TILES + t
                    nc.tensor.matmul(
                        st[f"av{h}"][:, q0:], lhsT=v_sb[:, idx, :],
                        rhs=pexp_sl(pexp, t, h, q0, QSTRIP),
                        start=(t == 0), stop=(t == T - 1))
            return u

        def mk_norm():
            def u():
                # copy AV out of PSUM fast (releases the banks); the
                # denominator row is broadcast across partitions by a tiny
                # K=1 PE matmul against a ones column, then inverted on DVE
                av_sb = sbp.tile([HD + 1, 2, QSTRIP], BF16, tag="avsb",
                                 name="av_sb")
                db = [None, None]
                for h in range(HLOC):
                    if h == 0:
                        nc.vector.tensor_copy(av_sb[:, h, :], st[f"av{h}"])
                    else:
                        nc.scalar.copy(av_sb[:, h, :], st[f"av{h}"])
                    db[h] = psM.tile([HD, QSTRIP], F32, tag="mm",
                                     name=f"db{h}")
                    nc.tensor.matmul(db[h],
                                     lhsT=ones_sb[HD:HD + 1, :],
                                     rhs=av_sb[HD:HD + 1, h, :],
                                     start=True, stop=True)
                rb = rbp.tile([HD, 2, QSTRIP], F32, tag="rb")
                for h in range(HLOC):
                    nc.vector.reciprocal_approx_fast(rb[:, h, :], db[h])
                nc.vector.tensor_mul(avf[g][0:HD, :], av_sb[0:HD, 0, :],
                                     rb[:, 0, :])
                avh = sbp.tile([HD, QSTRIP], BF16, tag="avh")
                nc.vector.tensor_mul(avh, av_sb[0:HD, 1, :], rb[:, 1, :])
                nc.gpsimd.dma_start(out=avf[g][HD:2 * HD, :], in_=avh)
            return u

        def mk_exp_fine(t):
            # last strip: just exp+mask; AV deferred to per-q-block units
            def u():
                if t == 0:
                    for h in range(HLOC):
                        st[f"avq{h}"] = psV.tile([HD + 1, 4, 128], F32,
                                                 tag="av", name=f"avq{h}")
                st[f"pexp{t}"] = mk_exp(t)
            return u

        def mk_av_fine(qb):
            # one contiguous accumulation group per q-block (interleaved
            # open groups in one PSUM bank corrupt results)
            def u():
                for h in range(HLOC):
                    for t in range(qb + 1):
                        idx = (b * HLOC + h) * KT_TILES + t
                        nc.tensor.matmul(
                            st[f"avq{h}"][:, qb, :], lhsT=v_sb[:, idx, :],
                            rhs=pexp_sl(st[f"pexp{t}"], t, h, qb * 128,
                                        (qb + 1) * 128),
                            start=(t == 0), stop=(t == qb))
            return u

        def mk_norm_fine(qb):
            def u():
                av_sbq = sbp.tile([HD + 1, 2, 128], BF16, tag="avsbq",
                                  name="av_sbq")
                for h in range(HLOC):
                    nc.vector.tensor_copy(av_sbq[:, h, :],
                                          st[f"avq{h}"][:, qb, :])
                db = psM.tile([HD, 2, 128], F32, tag="mm", name="dbq")
                for h in range(HLOC):
                    nc.tensor.matmul(db[:, h, :],
                                     lhsT=ones_sb[HD:HD + 1, :],
                                     rhs=av_sbq[HD:HD + 1, h, :],
                                     start=True, stop=True)
                rbq = rbp.tile([HD, 2, 128], F32, tag="rbq")
                nc.vector.reciprocal_approx_fast(rbq, db)
                qs = qb * 128
                nc.vector.tensor_mul(avf[g][0:HD, qs:qs + 128],
                                     av_sbq[0:HD, 0, :], rbq[:, 0, :])
                avhq = sbp.tile([HD, 128], BF16, tag="avhq")
                nc.vector.tensor_mul(avhq, av_sbq[0:HD, 1, :], rbq[:, 1, :])
                nc.gpsimd.dma_start(out=avf[g][HD:2 * HD, qs:qs + 128],
                                    in_=avhq)
            return u

        if fine_tail:
            assert T == 4
            outu = out_units(g)
            units = [mk_sc(0), mk_sc(1)]
            for t in range(T):
                units.append(mk_exp_fine(t))
                if t + 2 < T:
                    units.append(mk_sc(t + 2))
                units.append(mk_av_fine(t))
                units.append(mk_norm_fine(t))
                units.append(outu[t])
            return units

        units = [mk_sc(0), mk_sc(1)]
        for t in range(T):
            units.append(mk_ea(t))
            if t + 2 < T:
                units.append(mk_sc(t + 2))
        units.append(mk_norm())
        return units

    def out_units(g):
        gs = g * QSTRIP
        units = []
        def mk(tt):
            def u():
                ob = outp.tile([128, D], BF16, tag="ob")
                for n in range(2):
                    op_ps = psM.tile([128, 512], F32, tag="mm", name="op_ps")
                    nc.tensor.matmul(
                        op_ps, lhsT=avf[g][:, tt * 128:(tt + 1) * 128],
                        rhs=wo_sb[:, n * 512:(n + 1) * 512],
                        start=True, stop=True)
                    # gap-filler copy: Tile routes to whichever of ACT/DVE
                    # is idle when it becomes ready
                    nc.any.tensor_copy(ob[:, n * 512:(n + 1) * 512], op_ps)
                nc.sync.dma_start(
                    out=out[gs + tt * 128: gs + (tt + 1) * 128, :], in_=ob)
            return u
        for tt in range(4):
            units.append(mk(tt))
        return units

    # strip order: b0 ascending then b1 descending (short strip last).
    order = [0, 1, 2, 3, 7, 6, 5, 4]
    pu = {g: proj_units(g) for g in range(NSTRIP)}

    def pf(g, keys):
        return [u for k in keys for u in pu[g][k]]

    for u in pf(0, "qkv"):
        u()

    fill_sched = {
        0: lambda: pf(1, "qkv"),
        1: lambda: pf(2, "qkv") + out_units(0),
        2: lambda: pf(3, "qkv") + pf(4, "kv") + out_units(1),
        3: lambda: pf(7, "qkv") + pf(6, "kv") + pf(5, "kv")
                   + out_units(2),
        7: lambda: pf(6, "q"),
        6: lambda: out_units(3) + pf(5, "q"),
        5: lambda: out_units(7) + pf(4, "q"),
        4: None,
    }
    for g in order[:-1]:
        _interleave(attn_units(g), fill_sched[g]())
    ou5 = out_units(5)
    _interleave(attn_units(4), out_units(6) + ou5[0:1])
    # dummy warm-fill matmuls: execute during the final norm-chain stalls so
    # the HAM clock stays at full rate for the drain's real matmuls
    wfill = psV.tile([HD, QSTRIP], F32, tag="av", name="wfill")
    ou4 = out_units(4)
    tail = [ou5[1], ou5[2], ou5[3]] + ou4
    for i, u in enumerate(tail):
        for r in range(2):
            nc.tensor.matmul(wfill, lhsT=ones_sb[0:HD, :], rhs=wrm,
                             start=(i == 0 and r == 0),
                             stop=(i == len(tail) - 1 and r == 1))
        u()


_CACHED_NC = None


def build_module():
    global _CACHED_NC
    if _CACHED_NC is None:
        nc = bacc.Bacc("TRN2", debug=False)
        with tile.TileContext(nc) as tc:
            with ExitStack() as ctx:
                _build_kernel(ctx, tc)
        nc.compile()
        _CACHED_NC = nc
    return _CACHED_NC


def make_in_maps(x, Wq, Wk, Wv, Wo):
    x = np.asarray(x, np.float32)
    xTf = x.reshape(NT, D).T                        # [D, NT] fp32
    # device layout [p, strip, k, col]: row d = k*128 + p
    xT4 = xTf.reshape(8, 128, NSTRIP, QSTRIP).transpose(1, 2, 0, 3)
    xT = np.ascontiguousarray(xT4.astype(NPBF16))
    if FP8_QK:
        # fp8 copy for Q/K: [p, strip, kg, ko, col] with k = 2*kg + ko
        xq = np.ascontiguousarray(
            xT4.reshape(128, NSTRIP, 4, 2, QSTRIP).astype(NPFP8))
    # triangular keep mask for the diagonal 128-block, duplicated per head
    i = np.arange(128)[:, None]
    c = np.arange(128)[None, :]
    tri = (c >= i).astype(NPBF16)
    msk = np.ascontiguousarray(
        np.stack([tri, tri], axis=1))               # [128, 2, 128]
    in_maps = []
    for core in range(NCORES):
        cs = slice(core * CW, (core + 1) * CW)
        def warr8(W):  # [D, CW] -> [p, kg, ko, col], scaled x32, fp8
            a = np.asarray(W, np.float32)[:, cs] * WSCALE
            return np.ascontiguousarray(
                a.reshape(4, 2, 128, CW).transpose(2, 0, 1, 3).astype(NPFP8))
        def warr(W):  # [D, CW] -> [p, k, col] with d = k*128 + p
            a = np.asarray(W, np.float32)[:, cs].astype(NPBF16)
            return np.ascontiguousarray(
                a.reshape(8, 128, CW).transpose(1, 0, 2))
        m = {
            "xt": xT,
            "wq": warr8(Wq) if FP8_QK else warr(Wq),
            "wk": warr8(Wk) if FP8_QK else warr(Wk),
            "wv": warr(Wv),
            "wo": np.ascontiguousarray(np.asarray(Wo, np.float32)[cs, :]).astype(NPBF16),
            "msk": msk,
            "idn": np.eye(128, dtype=NPBF16),
        }
        if FP8_QK:
            m["xq"] = xq
        in_maps.append(m)
    return in_maps


def kernel(x, Wq, bq, Wk, bk, Wv, bv, Wo, bo):
    for b_ in (bq, bk, bv, bo):
        assert np.count_nonzero(np.asarray(b_)) == 0, "nonzero biases unsupported"
    nc = build_module()
    in_maps = make_in_maps(x, Wq, Wk, Wv, Wo)
    res = run_bass_kernel_spmd(nc, in_maps, core_ids=list(range(NCORES)))
    partials = [res.results[c]["out"] for c in range(NCORES)]
    total = np.sum(np.stack(partials, 0).astype(np.float32), axis=0)
    return total.reshape(B, S, D)
